# revision 16
# baseline (speedup 1.0000x reference)
"""MHA kernel for Trainium2, 8 NeuronCores — ACT-paced pipeline v2.

Problem: B=4, T=2048, D=1024, H=16, HD=64 fp32 multi-head attention
  qkv = x @ w_qkv ; attention per head ; out = y @ w_o

Sharding: core c handles batch b = c//2 and head-group g = c%2 (8 of the 16
heads). Each core computes its 8 heads' attention output projected through
the matching w_o row-slice, producing a partial [T, D] f16 output; the host
sums the two partials per batch (row-parallel output projection).

The kernel is paced by the scalar (ACT) engine: 256 exp instructions of
[128, 1024] at ~1146ns = ~293us. Everything else is scheduled to hide
underneath with zero ACT stalls:

  Global slot g (period ~2292ns = two exps), window w = g//16 = (pair, tb),
  i = g%16 (s-tile). Per slot, in emission (= program) order:
    PE : sc_B(g-1)            (B scores run in the shadow of exp_A(g))
    ACT: exp_A(g); exp_B(g-1) (back-to-back, the pacer)
    DVE: acc_A(g); acc_B(g-1) (f16 exp-sum accumulate, 2x mode)
    PE : yu_A(g-LAG) || yu_B(g-LAG-1)  (col-tiled concurrent pair)
    PE : sc_A(g+1)            (A scores in the shadow of exp_B(g-1))
    PE : fill chains (QKV / output projection), budget-paced

  A and B exps ping-pong on two PSUM score tiles; the B stream lags one
  slot so each tile's write window is the other tile's exp duration.
  Softmax denominators: gpsimd partition_all_reduce on the f16 exp-sum
  (no PE/ones-matmul, no separate broadcast), then DVE reciprocal and
  one [64,1024] normalize multiply per head into yt (f16).
"""
import sys

if "/opt/trn_rl_repo" not in sys.path:
    sys.path.insert(0, "/opt/trn_rl_repo")

from collections import deque

import numpy as np

import concourse.bass as bass
import concourse.mybir as mybir
import concourse.tile as tile
from concourse import bacc
from concourse.bass_isa import ReduceOp
from concourse.bass_utils import run_bass_kernel_spmd

T = 2048
D = 1024
NH = 8          # heads per core
HD = 64
KC = D // 128   # 8 contraction chunks
TT = T // 128   # 16 s tiles
NP = NH // 2    # 4 head pairs
NW = 2 * NP     # 8 windows: w = 2*p + tb
NG = NW * TT    # 128 global slots
LAG = 3         # yu lags exp by LAG slots
F32 = mybir.dt.float32
F16 = mybir.dt.float16

_CACHE = {}
_DEBUG = False


def build_nc():
    nc = bacc.Bacc(
        "TRN2",
        target_bir_lowering=False,
        debug=False,
        enable_asserts=False,
        num_devices=8,
    )
    x_d = nc.dram_tensor("x", [T, D], F16, kind="ExternalInput")
    wq_d = nc.dram_tensor("wq", [D, 512], F16, kind="ExternalInput")
    wk_d = nc.dram_tensor("wk", [D, 512], F16, kind="ExternalInput")
    wv_d = nc.dram_tensor("wv", [D, 512], F16, kind="ExternalInput")
    wo_d = nc.dram_tensor("wo", [512, D], F16, kind="ExternalInput")
    out_d = nc.dram_tensor("out", [T, D], F16, kind="ExternalOutput")
    if _DEBUG:
        qkt_d = nc.dram_tensor("qkt_dump", [128, 8, T], F16,
                               kind="ExternalOutput")
        v_d = nc.dram_tensor("v_dump", [128, TT, 512], F16,
                             kind="ExternalOutput")
        yt_d = nc.dram_tensor("yt_dump", [128, NP, T], F16,
                              kind="ExternalOutput")

    x_ap = x_d.ap()
    wq_ap = wq_d.ap().rearrange("(kc p) j -> p kc j", p=128)   # [128, 8, 512]
    wk_ap = wk_d.ap().rearrange("(kc p) j -> p kc j", p=128)
    wv_ap = wv_d.ap().rearrange("(kc p) j -> p kc j", p=128)
    wo_ap = wo_d.ap().rearrange("(c p) n -> p c n", p=128)     # [128, 4, 1024]

    def win(g):
        """global slot -> (pair, tb, i)."""
        w, i = g // TT, g % TT
        return w // 2, w % 2, i

    with tile.TileContext(nc) as tc:
        with (
            tc.sbuf_pool(name="sb", bufs=1) as sb,
            tc.psum_pool(name="ps", bufs=1) as ps,
        ):
            # ---- persistent sbuf ----
            xt = sb.tile([128, KC, T], F16)          # x^T  [d, t]
            qkt = sb.tile([128, 8, T], F16)          # jt 0-3 Q^T, 4-7 K^T
            v_sb = sb.tile([128, TT, 512], F16)      # V [s-part, s-chunk, j]
            yt = sb.tile([128, NP, T], F16)          # y^T [dy, pair, t]
            wqk_sb = sb.tile([128, KC, 1024], F16)   # cols 0-511 wq, 512+ wk
            wv_sb = sb.tile([128, KC, 512], F16)
            wo_sb = sb.tile([128, 4, D], F16)
            warm = sb.tile([1, 32], F16)
            nc.vector.memset(warm, 0.0)
            # warm up the ACT exp table before the stream needs it
            nc.scalar.activation(
                warm, warm, mybir.ActivationFunctionType.Exp, scale=0.125
            )

            nc.sync.dma_start(out=wqk_sb[:, :, 512:1024], in_=wk_ap)

            # ---------- fill chains (QKV / O projections) ----------
            fills = deque()
            pending = {}

            def g_qk(jt, tbc):
                """qkt[:, jt, tbc*512:(tbc+1)*512] = (w chunk)^T @ xt."""
                aux = ps.tile([128, 512], F32, name="qkps", tag="aux", bufs=2)
                for kc in range(KC):
                    nc.tensor.matmul(
                        aux,
                        wqk_sb[:, kc, jt * 128:(jt + 1) * 128],
                        xt[:, kc, tbc * 512:(tbc + 1) * 512],
                        start=(kc == 0),
                        stop=(kc == KC - 1),
                        skip_group_check=True,
                    )
                    yield 230
                nc.vector.tensor_copy(
                    out=qkt[:, jt, tbc * 512:(tbc + 1) * 512], in_=aux
                )

            def g_v(i):
                aux = ps.tile([128, 512], F32, name="vps", tag="aux", bufs=2)
                for kc in range(KC):
                    nc.tensor.matmul(
                        aux,
                        xt[:, kc, i * 128:(i + 1) * 128],
                        wv_sb[:, kc, :],
                        start=(kc == 0),
                        stop=(kc == KC - 1),
                        skip_group_check=True,
                    )
                    yield 230
                nc.vector.tensor_copy(out=v_sb[:, i, :], in_=aux)

            def g_o(tt, u):
                aux = ps.tile([128, 512], F32, name="ops", tag="aux", bufs=2)
                for c4 in range(4):
                    nc.tensor.matmul(
                        aux,
                        yt[:, c4, tt * 128:(tt + 1) * 128],
                        wo_sb[:, c4, u * 512:(u + 1) * 512],
                        start=(c4 == 0),
                        stop=(c4 == 3),
                        skip_group_check=True,
                    )
                    yield 230
                o_sb = sb.tile([128, 512], F16, tag="osb", bufs=2)
                with nc.allow_low_precision(reason="f16 partial output"):
                    nc.vector.tensor_copy(out=o_sb, in_=aux)
                nc.sync.dma_start(
                    out=out_d.ap()[
                        tt * 128:(tt + 1) * 128, u * 512:(u + 1) * 512
                    ],
                    in_=o_sb,
                )

            def push_fill(key, gen):
                pending[key] = gen
                fills.append(key)

            def advance_fills(budget):
                while fills and budget > 0:
                    gen = pending.get(fills[0])
                    if gen is None:
                        fills.popleft()
                        continue
                    try:
                        budget -= next(gen)
                    except StopIteration:
                        del pending[fills[0]]
                        fills.popleft()

            def need(key):
                gen = pending.pop(key, None)
                if gen is not None:
                    for _ in gen:
                        pass

            def force_chain(gen):
                for _ in gen:
                    pass

            # ---------- attention state ----------
            sc_t = {
                "A": ps.tile([128, 1024], F32, name="sca", tag="sca", bufs=1),
                "B": ps.tile([128, 1024], F32, name="scb", tag="scb", bufs=1),
            }
            yu = ps.tile([128, 1024], F32, name="yu", tag="yu", bufs=1)
            exp_t = {}     # (h, g) -> sbuf exp tile
            acc_t = {}     # h -> running f16 exp-sum tile
            ar_t = {}      # w -> denominator all-reduce tile (A|B halves)
            bc_t = {}      # w -> reciprocal-denominator tile (A|B halves)

            def head_slices(p, h):
                """(stationary partition range, v column base) for head h."""
                pb = 0 if h == "A" else 64
                return pb, 128 * p + pb

            def emit_sc(h, g):
                """scores for head h of window(g), s-tile i, into sc_t[h]."""
                if g < 0 or g >= NG:
                    return
                p, tb, i = win(g)
                pb, _ = head_slices(p, h)
                # make sure producer chains are done (memoized pops)
                need(("qk", 4 + p, i // 4))
                need(("qk", p, 2 * tb))
                need(("qk", p, 2 * tb + 1))
                for u in range(2):
                    nc.tensor.matmul(
                        sc_t[h][:, u * 512:(u + 1) * 512],
                        qkt[pb:pb + 64, 4 + p, i * 128:(i + 1) * 128],
                        qkt[pb:pb + 64, p,
                            tb * 1024 + u * 512:tb * 1024 + (u + 1) * 512],
                        start=True,
                        stop=True,
                    )

            def emit_exp(h, g):
                if g < 0 or g >= NG:
                    return
                e = sb.tile([128, 1024], F16, tag="exp" + h, bufs=5)
                nc.scalar.activation(
                    e, sc_t[h], mybir.ActivationFunctionType.Exp, scale=0.125
                )
                exp_t[(h, g)] = e

            def emit_acc(h, g):
                if g < 0 or g >= NG:
                    return
                i = g % TT
                a = sb.tile([128, 1024], F16, tag="acc" + h, bufs=3)
                if i == 0:
                    nc.vector.tensor_copy(out=a, in_=exp_t[(h, g)])
                else:
                    with nc.allow_low_precision(reason="f16 exp-sum"):
                        nc.vector.tensor_add(
                            out=a, in0=acc_t[h], in1=exp_t[(h, g)]
                        )
                acc_t[h] = a
                if i == TT - 1:
                    # denominator chain depends only on the finished acc;
                    # start it now so gpsimd+recip finish well before the
                    # normalize multiply needs them. A and B share one
                    # [128, 2048] tile so a single full-width reciprocal
                    # covers both (reciprocal_approx_fast requires
                    # partition base 0).
                    w = g // TT
                    if h == "A":
                        ar = sb.tile([128, 2048], F32, tag="ar", bufs=2)
                        ar_t[w] = ar
                        nc.gpsimd.partition_all_reduce(
                            ar[:, 0:1024], a, channels=128,
                            reduce_op=ReduceOp.add,
                        )
                    else:
                        ar = ar_t.pop(w)
                        nc.gpsimd.partition_all_reduce(
                            ar[:, 1024:2048], a, channels=128,
                            reduce_op=ReduceOp.add,
                        )
                        bc = sb.tile([128, 2048], F32, tag="bc", bufs=2)
                        nc.vector.reciprocal_approx_fast(out=bc, in_=ar)
                        bc_t[w] = bc

            def emit_yu(hA_g, hB_g):
                """col-tiled concurrent pair: yu_A(hA_g) || yu_B(hB_g)."""
                for h, g in (("A", hA_g), ("B", hB_g)):
                    if g is None or g < 0 or g >= NG:
                        continue
                    p, tb, i = win(g)
                    w = g // TT
                    pb, vcol = head_slices(p, h)
                    need(("v", i))
                    e = exp_t[(h, g)]
                    for u in range(2):
                        nc.tensor.matmul(
                            yu[pb:pb + 64, u * 512:(u + 1) * 512],
                            v_sb[:, i, vcol:vcol + 64],
                            e[:, u * 512:(u + 1) * 512],
                            start=(i == 0),
                            stop=(i == TT - 1),
                            skip_group_check=True,
                        )
                    exp_t.pop((h, g))
                    if i == TT - 1:
                        # head-h accumulation complete: normalize into yt
                        hb = 0 if h == "A" else 1
                        bc = bc_t[w] if h == "B" else bc_t.get(w)
                        assert bc is not None
                        if h == "B":
                            bc_t.pop(w)
                        with nc.allow_low_precision(reason="f16 y"):
                            nc.vector.tensor_mul(
                                out=yt[pb:pb + 64, p,
                                       tb * 1024:(tb + 1) * 1024],
                                in0=yu[pb:pb + 64, :],
                                in1=bc[pb:pb + 64,
                                       1024 * hb:1024 * (hb + 1)],
                            )

            # ---------- startup ----------
            for kc in range(KC):
                nc.sync.dma_start_transpose(
                    out=xt[:, kc, 0:1024],
                    in_=x_ap[0:1024, kc * 128:(kc + 1) * 128],
                )
            nc.sync.dma_start(out=wqk_sb[:, :, 0:512], in_=wq_ap)
            force_chain(g_qk(4, 0))   # K^T pair 0, s 0:512
            force_chain(g_qk(0, 0))   # Q^T pair 0, t 0:512
            force_chain(g_qk(0, 1))   # Q^T pair 0, t 512:1024
            nc.sync.dma_start(out=wv_sb, in_=wv_ap)
            v_gens = {i: g_v(i) for i in range(TT)}
            force_chain(v_gens.pop(0))
            force_chain(v_gens.pop(1))

            def xpose2(kc):
                nc.sync.dma_start_transpose(
                    out=xt[:, kc, 1024:2048],
                    in_=x_ap[1024:2048, kc * 128:(kc + 1) * 128],
                )

            startup_forced = {
                0: [lambda: [xpose2(kc) for kc in range(4)]],
                1: [lambda: [xpose2(kc) for kc in range(4, KC)],
                    lambda: force_chain(g_qk(4, 1))],
                2: [lambda: nc.sync.dma_start(out=wo_sb, in_=wo_ap)],
                4: [lambda: force_chain(g_qk(4, 2))],
                6: [lambda: force_chain(g_qk(4, 3))],
            }

            def fill_pushes(g):
                """push new fill chains at window starts."""
                w, i = g // TT, g % TT
                if i == 0:
                    p, tb = w // 2, w % 2
                    if p < 3:
                        jt = (p + 1) if tb == 0 else (4 + p + 1)
                        for tbc in range(4):
                            push_fill(("qk", jt, tbc), g_qk(jt, tbc))
                    if w == 0:
                        push_fill(("qk", 0, 2), g_qk(0, 2))
                        push_fill(("qk", 0, 3), g_qk(0, 3))
                # o(tb0) chains read yt pair-3 tb0, whose B-half norm is
                # emitted at slot 16*6+15+LAG+1; push strictly after it.
                if g == 16 * 6 + 15 + LAG + 2:
                    for tt in range(8):
                        for u in range(2):
                            push_fill(("o", tt, u), g_o(tt, u))

            # v chains keyed for need(); remaining ones stream in window 0
            for i_ in sorted(v_gens):
                pending[("v", i_)] = v_gens[i_]
                fills.append(("v", i_))

            # sc_A(0) must exist before exp_A(0)
            emit_sc("A", 0)

            # ---------- main loop ----------
            for g in range(NG + LAG + 2):
                first_win = g < TT
                if g < NG:
                    fill_pushes(g)
                emit_sc("B", g - 1)
                emit_exp("A", g)
                emit_exp("B", g - 1)
                emit_acc("A", g)
                emit_acc("B", g - 1)
                emit_yu(g - LAG, None)
                emit_yu(None, g - LAG - 1)
                if first_win:
                    for fn in startup_forced.get(g, ()):
                        fn()
                advance_fills(200 if first_win else 430)
                emit_sc("A", g + 1)
                advance_fills(200 if first_win else 430)

            # ---------- tail: output projection for tb=1 ----------
            while fills:
                advance_fills(10000)
            for tt in range(8, 16):
                for u in range(2):
                    force_chain(g_o(tt, u))
            if _DEBUG:
                nc.sync.dma_start(out=qkt_d.ap(), in_=qkt)
                nc.sync.dma_start(out=v_d.ap(), in_=v_sb)
                nc.sync.dma_start(out=yt_d.ap(), in_=yt)

    nc.compile()
    return nc


def make_in_maps(x, w_qkv, w_o):
    in_maps = []
    for c in range(8):
        b, gg = c // 2, c % 2
        in_maps.append({
            "x": np.ascontiguousarray(x[b], dtype=np.float16),
            "wq": np.ascontiguousarray(
                w_qkv[:, 512 * gg:512 * (gg + 1)], dtype=np.float16),
            "wk": np.ascontiguousarray(
                w_qkv[:, 1024 + 512 * gg:1024 + 512 * (gg + 1)],
                dtype=np.float16),
            "wv": np.ascontiguousarray(
                w_qkv[:, 2048 + 512 * gg:2048 + 512 * (gg + 1)],
                dtype=np.float16),
            "wo": np.ascontiguousarray(
                w_o[512 * gg:512 * (gg + 1), :], dtype=np.float16),
        })
    return in_maps


def kernel(x, w_qkv, w_o, _trace=False, _trace_kwargs=None):
    x = np.asarray(x)
    w_qkv = np.asarray(w_qkv)
    w_o = np.asarray(w_o)
    if "nc" not in _CACHE:
        _CACHE["nc"] = build_nc()
    nc = _CACHE["nc"]
    in_maps = make_in_maps(x, w_qkv, w_o)
    res = run_bass_kernel_spmd(
        nc, in_maps, core_ids=list(range(8)),
        trace=_trace, **(_trace_kwargs or {}),
    )
    out = np.empty((4, T, D), np.float32)
    for b in range(4):
        out[b] = (res.results[2 * b]["out"].astype(np.float32)
                  + res.results[2 * b + 1]["out"].astype(np.float32))
    if _trace:
        _CACHE["last_res"] = res
    return out


# revision 17
# speedup vs baseline: 1.0639x; 1.0639x over previous
"""MHA kernel for Trainium2, 8 NeuronCores — ACT-paced pipeline v3.

Problem: B=4, T=2048, D=1024, H=16, HD=64 fp32 multi-head attention
  qkv = x @ w_qkv ; attention per head ; out = y @ w_o

Sharding: core c handles batch b = c//2 and head-group g = c%2 (8 of the 16
heads). Each core computes its 8 heads' attention output projected through
the matching w_o row-slice, producing a partial [T, D] f16 output; the host
sums the two partials per batch (row-parallel output projection).

Pacing: the scalar (ACT) engine runs ONE combined [128, 2048] exp per slot
(A-head half | B-head half, (2048+352)/1.2 = 2000ns) — 128 slots = 256us.
Per slot g (window w = g//16 = (pair, tb), s-tile i = g%16):

  ACT: exp(g)           reads sc [128,2048] psum (A|B), written slot g-1
  DVE: acc(g) += e(g)   f16 exp-sum, one [128,2048] 2x-mode add
  PE : yu pairs (g-LAG) col-tiled A||B concurrent, into yu [128,1024]
  PE : fills            QKV / O projection chains, budget-paced
  PE : scores(g+1)      4 MMs as 2 row-tiled A||B concurrent pairs
                        (WAR: must follow exp(g) — the only ACT stall)
  PE : fills

At i==15: denominator = ones-matmul of acc in 4 [1,512] chunks through the
aux psum bank -> DVE reciprocal into rec[1,2048] -> gpsimd
partition_broadcast to bc[128,2048] -> one [64,1024] normalize multiply per
head into yt (f16) when that head's yu finishes (LAG slots later).
"""
import sys

if "/opt/trn_rl_repo" not in sys.path:
    sys.path.insert(0, "/opt/trn_rl_repo")

from collections import deque

import numpy as np

import concourse.bass as bass
import concourse.mybir as mybir
import concourse.tile as tile
from concourse import bacc
from concourse.bass_utils import run_bass_kernel_spmd

T = 2048
D = 1024
NH = 8          # heads per core
HD = 64
KC = D // 128   # 8 contraction chunks
TT = T // 128   # 16 s tiles
NP = NH // 2    # 4 head pairs
NW = 2 * NP     # 8 windows: w = 2*p + tb
NG = NW * TT    # 128 global slots
LAG = 4         # yu lags exp by LAG slots
F32 = mybir.dt.float32
F16 = mybir.dt.float16

_CACHE = {}
_DEBUG = False


def build_nc():
    nc = bacc.Bacc(
        "TRN2",
        target_bir_lowering=False,
        debug=False,
        enable_asserts=False,
        num_devices=8,
    )
    x_d = nc.dram_tensor("x", [T, D], F16, kind="ExternalInput")
    wq_d = nc.dram_tensor("wq", [D, 512], F16, kind="ExternalInput")
    wk_d = nc.dram_tensor("wk", [D, 512], F16, kind="ExternalInput")
    wv_d = nc.dram_tensor("wv", [D, 512], F16, kind="ExternalInput")
    wo_d = nc.dram_tensor("wo", [512, D], F16, kind="ExternalInput")
    out_d = nc.dram_tensor("out", [T, D], F16, kind="ExternalOutput")
    if _DEBUG:
        qkt_d = nc.dram_tensor("qkt_dump", [128, 8, T], F16,
                               kind="ExternalOutput")
        v_d = nc.dram_tensor("v_dump", [128, TT, 512], F16,
                             kind="ExternalOutput")
        yt_d = nc.dram_tensor("yt_dump", [128, NP, T], F16,
                              kind="ExternalOutput")

    x_ap = x_d.ap()
    wq_ap = wq_d.ap().rearrange("(kc p) j -> p kc j", p=128)   # [128, 8, 512]
    wk_ap = wk_d.ap().rearrange("(kc p) j -> p kc j", p=128)
    wv_ap = wv_d.ap().rearrange("(kc p) j -> p kc j", p=128)
    wo_ap = wo_d.ap().rearrange("(c p) n -> p c n", p=128)     # [128, 4, 1024]

    def win(g):
        """global slot -> (pair, tb, i)."""
        w, i = g // TT, g % TT
        return w // 2, w % 2, i

    with tile.TileContext(nc) as tc:
        with (
            tc.sbuf_pool(name="sb", bufs=1) as sb,
            tc.psum_pool(name="ps", bufs=1) as ps,
        ):
            # ---- persistent sbuf ----
            xt = sb.tile([128, KC, T], F16)          # x^T  [d, t]
            qkt = sb.tile([128, 8, T], F16)          # jt 0-3 Q^T, 4-7 K^T
            v_sb = sb.tile([128, TT, 512], F16)      # V [s-part, s-chunk, j]
            yt = sb.tile([128, NP, T], F16)          # y^T [dy, pair, t]
            wqk_sb = sb.tile([128, KC, 1024], F16)   # cols 0-511 wq, 512+ wk
            wv_sb = sb.tile([128, KC, 512], F16)
            wo_sb = sb.tile([128, 4, D], F16)
            ones_v = sb.tile([128, 1], F16)
            nc.vector.memset(ones_v, 1.0)
            warm = sb.tile([1, 32], F16)
            nc.vector.memset(warm, 0.0)
            # warm up the ACT exp table before the stream needs it
            nc.scalar.activation(
                warm, warm, mybir.ActivationFunctionType.Exp, scale=0.125
            )

            nc.sync.dma_start(out=wqk_sb[:, :, 512:1024], in_=wk_ap)

            # ---------- fill chains (QKV / O projections) ----------
            fills = deque()
            pending = {}

            def g_qk(jt, tbc):
                """qkt[:, jt, tbc*512:(tbc+1)*512] = (w chunk)^T @ xt."""
                aux = ps.tile([128, 512], F32, name="qkps", tag="aux", bufs=2)
                for kc in range(KC):
                    nc.tensor.matmul(
                        aux,
                        wqk_sb[:, kc, jt * 128:(jt + 1) * 128],
                        xt[:, kc, tbc * 512:(tbc + 1) * 512],
                        start=(kc == 0),
                        stop=(kc == KC - 1),
                        skip_group_check=True,
                    )
                    yield 230
                nc.vector.tensor_copy(
                    out=qkt[:, jt, tbc * 512:(tbc + 1) * 512], in_=aux
                )

            def g_v(i):
                aux = ps.tile([128, 512], F32, name="vps", tag="aux", bufs=2)
                for kc in range(KC):
                    nc.tensor.matmul(
                        aux,
                        xt[:, kc, i * 128:(i + 1) * 128],
                        wv_sb[:, kc, :],
                        start=(kc == 0),
                        stop=(kc == KC - 1),
                        skip_group_check=True,
                    )
                    yield 230
                nc.vector.tensor_copy(out=v_sb[:, i, :], in_=aux)

            def g_o(tt, u):
                aux = ps.tile([128, 512], F32, name="ops", tag="aux", bufs=2)
                for c4 in range(4):
                    nc.tensor.matmul(
                        aux,
                        yt[:, c4, tt * 128:(tt + 1) * 128],
                        wo_sb[:, c4, u * 512:(u + 1) * 512],
                        start=(c4 == 0),
                        stop=(c4 == 3),
                        skip_group_check=True,
                    )
                    yield 230
                o_sb = sb.tile([128, 512], F16, tag="osb", bufs=2)
                with nc.allow_low_precision(reason="f16 partial output"):
                    nc.vector.tensor_copy(out=o_sb, in_=aux)
                nc.sync.dma_start(
                    out=out_d.ap()[
                        tt * 128:(tt + 1) * 128, u * 512:(u + 1) * 512
                    ],
                    in_=o_sb,
                )

            def push_fill(key, gen):
                pending[key] = gen
                fills.append(key)

            def advance_fills(budget):
                while fills and budget > 0:
                    gen = pending.get(fills[0])
                    if gen is None:
                        fills.popleft()
                        continue
                    try:
                        budget -= next(gen)
                    except StopIteration:
                        del pending[fills[0]]
                        fills.popleft()

            def need(key):
                gen = pending.pop(key, None)
                if gen is not None:
                    for _ in gen:
                        pass

            def force_chain(gen):
                for _ in gen:
                    pass

            # ---------- attention state ----------
            # sc: A-half cols 0:1024 (banks 0-1), B-half 1024:2048 (banks 2-3)
            sc = ps.tile([128, 2048], F32, name="sc", tag="sc", bufs=1)
            yu = ps.tile([128, 1024], F32, name="yu", tag="yu", bufs=1)
            exp_t = {}     # g -> sbuf exp tile [128, 2048] (A|B)
            acc_t = [None]  # running f16 exp-sum tile [128, 2048]
            bc_t = {}      # w -> broadcast 1/denominator tile [128, 2048]

            def emit_sc(g):
                """scores for slot g: 4 MMs as 2 row-tiled A||B pairs."""
                if g < 0 or g >= NG:
                    return
                p, tb, i = win(g)
                need(("qk", 4 + p, i // 4))
                need(("qk", p, 2 * tb))
                need(("qk", p, 2 * tb + 1))
                for u in range(2):
                    for hb in range(2):   # A then B adjacent -> concurrent
                        pb = 64 * hb
                        nc.tensor.matmul(
                            sc[:, 1024 * hb + u * 512:
                               1024 * hb + (u + 1) * 512],
                            qkt[pb:pb + 64, 4 + p, i * 128:(i + 1) * 128],
                            qkt[pb:pb + 64, p,
                                tb * 1024 + u * 512:tb * 1024 + (u + 1) * 512],
                            start=True,
                            stop=True,
                        )

            def emit_exp(g):
                if g < 0 or g >= NG:
                    return
                e = sb.tile([128, 2048], F16, tag="exp", bufs=6)
                nc.scalar.activation(
                    e, sc, mybir.ActivationFunctionType.Exp, scale=0.125
                )
                exp_t[g] = e

            def emit_acc(g):
                if g < 0 or g >= NG:
                    return
                i = g % TT
                a = sb.tile([128, 2048], F16, tag="acc", bufs=2)
                if i == 0:
                    nc.vector.tensor_copy(out=a, in_=exp_t[g])
                else:
                    with nc.allow_low_precision(reason="f16 exp-sum"):
                        nc.vector.tensor_add(out=a, in0=acc_t[0], in1=exp_t[g])
                acc_t[0] = a
                if i == TT - 1:
                    # denominator: 4x [1,512] ones-matmul chunks via aux,
                    # reciprocal into rec, broadcast to bc.
                    w = g // TT
                    rec = sb.tile([1, 2048], F32, tag="rec", bufs=1)
                    bc = sb.tile([128, 2048], F32, tag="bc", bufs=1)
                    for c in range(4):
                        dn = ps.tile([128, 512], F32, name="dn",
                                     tag="aux", bufs=2)
                        nc.tensor.matmul(
                            dn[0:1, :],
                            ones_v,
                            a[:, c * 512:(c + 1) * 512],
                            start=True,
                            stop=True,
                            tile_position=(0, 0),
                        )
                        nc.vector.reciprocal_approx_fast(
                            out=rec[0:1, c * 512:(c + 1) * 512],
                            in_=dn[0:1, :],
                        )
                        nc.gpsimd.partition_broadcast(
                            bc[:, c * 512:(c + 1) * 512],
                            rec[0:1, c * 512:(c + 1) * 512],
                            channels=128,
                        )
                    bc_t[w] = bc

            def emit_yu(g):
                """col-tiled concurrent pairs: yu_A(g) || yu_B(g)."""
                if g < 0 or g >= NG:
                    return
                p, tb, i = win(g)
                w = g // TT
                need(("v", i))
                e = exp_t.pop(g)
                for u in range(2):
                    for hb in range(2):   # A then B adjacent -> concurrent
                        pb = 64 * hb
                        nc.tensor.matmul(
                            yu[pb:pb + 64, u * 512:(u + 1) * 512],
                            v_sb[:, i, 128 * p + pb:128 * p + pb + 64],
                            e[:, 1024 * hb + u * 512:1024 * hb + (u + 1) * 512],
                            start=(i == 0),
                            stop=(i == TT - 1),
                            skip_group_check=True,
                        )
                if i == TT - 1:
                    # normalize both heads into yt
                    bc = bc_t.pop(w)
                    with nc.allow_low_precision(reason="f16 y"):
                        for hb in range(2):
                            pb = 64 * hb
                            nc.vector.tensor_mul(
                                out=yt[pb:pb + 64, p,
                                       tb * 1024:(tb + 1) * 1024],
                                in0=yu[pb:pb + 64, :],
                                in1=bc[pb:pb + 64,
                                       1024 * hb:1024 * (hb + 1)],
                            )

            # ---------- startup ----------
            for kc in range(KC):
                nc.sync.dma_start_transpose(
                    out=xt[:, kc, 0:1024],
                    in_=x_ap[0:1024, kc * 128:(kc + 1) * 128],
                )
            nc.sync.dma_start(out=wqk_sb[:, :, 0:512], in_=wq_ap)
            force_chain(g_qk(4, 0))   # K^T pair 0, s 0:512
            force_chain(g_qk(0, 0))   # Q^T pair 0, t 0:512
            force_chain(g_qk(0, 1))   # Q^T pair 0, t 512:1024
            nc.sync.dma_start(out=wv_sb, in_=wv_ap)
            v_gens = {i: g_v(i) for i in range(TT)}
            for i_ in range(4):
                force_chain(v_gens.pop(i_))

            def xpose2(kc):
                nc.sync.dma_start_transpose(
                    out=xt[:, kc, 1024:2048],
                    in_=x_ap[1024:2048, kc * 128:(kc + 1) * 128],
                )

            startup_forced = {
                0: [lambda: [xpose2(kc) for kc in range(4)]],
                1: [lambda: [xpose2(kc) for kc in range(4, KC)],
                    lambda: force_chain(g_qk(4, 1))],
                2: [lambda: nc.sync.dma_start(out=wo_sb, in_=wo_ap)],
                4: [lambda: force_chain(g_qk(4, 2))],
                6: [lambda: force_chain(g_qk(4, 3))],
            }

            def fill_pushes(g):
                """push new fill chains at window starts."""
                w, i = g // TT, g % TT
                if i == 0:
                    p, tb = w // 2, w % 2
                    if p < 3:
                        jt = (p + 1) if tb == 0 else (4 + p + 1)
                        for tbc in range(4):
                            push_fill(("qk", jt, tbc), g_qk(jt, tbc))
                    if w == 0:
                        push_fill(("qk", 0, 2), g_qk(0, 2))
                        push_fill(("qk", 0, 3), g_qk(0, 3))
                # o(tb0) chains read yt pair-3 tb0, normalized at slot
                # 16*6+15+LAG; push strictly after.
                if g == 16 * 6 + 15 + LAG + 1:
                    for tt in range(8):
                        for u in range(2):
                            push_fill(("o", tt, u), g_o(tt, u))

            # v chains keyed for need(); stream as priority fills
            for i_ in sorted(v_gens):
                pending[("v", i_)] = v_gens[i_]
                fills.append(("v", i_))

            # sc(0) must exist before exp(0)
            emit_sc(0)

            # ---------- main loop ----------
            for g in range(NG + LAG + 1):
                first_win = g < TT
                if g < NG:
                    fill_pushes(g)
                emit_exp(g)
                emit_acc(g)
                emit_yu(g - LAG)
                if first_win:
                    for fn in startup_forced.get(g, ()):
                        fn()
                advance_fills(250 if first_win else 420)
                emit_sc(g + 1)
                advance_fills(250 if first_win else 430)

            # ---------- tail: output projection for tb=1 ----------
            while fills:
                advance_fills(10000)
            for tt in range(8, 16):
                for u in range(2):
                    force_chain(g_o(tt, u))
            if _DEBUG:
                nc.sync.dma_start(out=qkt_d.ap(), in_=qkt)
                nc.sync.dma_start(out=v_d.ap(), in_=v_sb)
                nc.sync.dma_start(out=yt_d.ap(), in_=yt)

    nc.compile()
    return nc


def make_in_maps(x, w_qkv, w_o):
    in_maps = []
    for c in range(8):
        b, gg = c // 2, c % 2
        in_maps.append({
            "x": np.ascontiguousarray(x[b], dtype=np.float16),
            "wq": np.ascontiguousarray(
                w_qkv[:, 512 * gg:512 * (gg + 1)], dtype=np.float16),
            "wk": np.ascontiguousarray(
                w_qkv[:, 1024 + 512 * gg:1024 + 512 * (gg + 1)],
                dtype=np.float16),
            "wv": np.ascontiguousarray(
                w_qkv[:, 2048 + 512 * gg:2048 + 512 * (gg + 1)],
                dtype=np.float16),
            "wo": np.ascontiguousarray(
                w_o[512 * gg:512 * (gg + 1), :], dtype=np.float16),
        })
    return in_maps


def kernel(x, w_qkv, w_o, _trace=False, _trace_kwargs=None):
    x = np.asarray(x)
    w_qkv = np.asarray(w_qkv)
    w_o = np.asarray(w_o)
    if "nc" not in _CACHE:
        _CACHE["nc"] = build_nc()
    nc = _CACHE["nc"]
    in_maps = make_in_maps(x, w_qkv, w_o)
    res = run_bass_kernel_spmd(
        nc, in_maps, core_ids=list(range(8)),
        trace=_trace, **(_trace_kwargs or {}),
    )
    out = np.empty((4, T, D), np.float32)
    for b in range(4):
        out[b] = (res.results[2 * b]["out"].astype(np.float32)
                  + res.results[2 * b + 1]["out"].astype(np.float32))
    if _trace:
        _CACHE["last_res"] = res
    return out
